# revision 22
# baseline (speedup 1.0000x reference)
"""Trainium2 Bass kernel for the 2-graph GCN (nn_Net_39041252721058).

Strategy (8 NeuronCores, SPMD single program):
  - Core k owns dst nodes [k*6250, (k+1)*6250). All edges with dst in that
    range are processed by core k, grouped by 128-node dst blocks.
  - Layer math uses the linearity of spmm: project first (x@W1 row-sharded,
    AllGather of the projected bf16 table), then per-edge gather rows of the
    table with SWDGE dma_gather (int16 indices; src split in two classes at
    row 32768 so indices fit), then segment-sum via one-hot matmuls on the
    tensor engine accumulating in PSUM (edges on the contraction axis).
  - The one-hot dst-slot selection matrices (sel = onehot(slot)*w) are built
    per block on the vector engine (is_equal + mult) from compact packed
    (slot, w) arrays; shipping pre-expanded sel from DRAM was tried and is
    net-negative: it loads the SDMA engines (~41 ns/descriptor, the
    co-bottleneck) with 99.3%-zeros traffic.
  - h = relu(agg + b1) on the scalar engine; the PE tail (transposes + h@W2)
    for block b is deferred until after block b+1's sel matmuls so the PE
    stream stays continuous (p-state ramps to 2.4 GHz only after ~3 us of
    uninterrupted execution; gaps reset it to 0.65/1.2 GHz).
  - Phases: A_d, AG1_d, A_s (hides AG1_d), AG1_s, B_d (hides AG1_s), AG2_d,
    B_s (hides AG2_d), AG2_s, C_d (hides AG2_s), C_s.

Perf model (measured on trn2, see git-less history in this session):
  - The kernel is bound by the per-edge gather stream: 2 graphs x 2 layers
    x 200K edges/core = ~827K gathered rows/core (incl. ~3.5% ceil-16 pad).
  - SWDGE descriptor generation runs ~8.4 ns/row per Q7 core pair. It is
    parallelized across all 4 SWDGE queues (bass num_swdge_queues=4; queue q
    runs on Q7 cores 2q,2q+1 with its own full-size descriptor ring in its
    own 32-partition group). Measured queue scaling on an isolated
    microbench (bench_queues.py): 1q=9.1, 2q=5.0, 4q=3.0 ns/row -- the 4q
    cap is partly SDMA-side (per-descriptor processing ~25-41 ns across 16
    engines; 512B rows are descriptor-dominated, 1KB rows 34% cheaper/byte).
  - Gather calls are chunked to 768 rows so two calls fit in a queue's
    descriptor ring (the NX await_space blocks in-order dispatch when a
    ring is full); call sizes are load-balanced across queues host-side
    (greedy least-loaded; naive round-robin gave a 2x per-queue imbalance).
  - Each call's idx slice is stored only in its queue's 32-partition window
    (4 queues overlay in the same columns), shrinking resident idx SBUF 4x.
  - num_idxs registers are hoisted (one per distinct call size) so gather
    dispatch does not pay a MOVE per call.
  - msgs tiles are 5-deep and sel/psum 3-deep so gathers run ~5 blocks
    ahead of the PE; block-level d/s interleaving was tried and is neutral
    (shared buffer pool halves per-graph lookahead).
  - Single-run best: 2.95 ms (baseline 8.76 ms). Remaining time: ~0.25 ms
    phase-A+AG1 head (AllGather chunking fails: a Shared DRAM tensor allows
    only one writer instruction), and B/C run ~10% above the isolated
    gather floor (cross-engine semaphore latency).

Correctness details: pads carry sel=0 so the one-hot multiply zeroes them;
message buffers are memset once on first use so never-gathered pad slots
cannot inject NaN (0*NaN = NaN would poison PSUM); phase-C reuses the same
buffers after phase B filled them with finite values, so only the first 5
phase-B allocations are memset.
"""
import numpy as np
import ml_dtypes

import concourse.bass as bass
import concourse.bacc as bacc
import concourse.mybir as mybir
import concourse.tile as tile
from concourse.bass_utils import run_bass_kernel_spmd

NCORES = 8
NODES = 50000
PER_CORE = NODES // NCORES           # 6250
NBLK = (PER_CORE + 127) // 128       # 49 (last block has 106 nodes)
SPLIT = 32768                        # int16 gather-index class boundary
F_IN = 512
H1 = 256
H2 = 128

BF16 = ml_dtypes.bfloat16


# ----------------------------------------------------------------------------
# Host-side edge preprocessing
# ----------------------------------------------------------------------------

def _analyze(src, dst):
    core = dst // PER_CORE
    blk = (dst % PER_CORE) // 128
    cls = (src >= SPLIT).astype(np.int64)
    key = (core * NBLK + blk) * 2 + cls
    counts = np.bincount(key, minlength=NCORES * NBLK * 2).reshape(-1, 2)
    return int(counts[:, 0].max()), int(counts[:, 1].max())


def _analyze_blocks(src, dst):
    """Per-block max-over-cores counts, ceil-16, per class: [NBLK, 2] int."""
    core = dst // PER_CORE
    blk = (dst % PER_CORE) // 128
    cls = (src >= SPLIT).astype(np.int64)
    key = (core * NBLK + blk) * 2 + cls
    counts = np.bincount(key, minlength=NCORES * NBLK * 2)
    counts = counts.reshape(NCORES, NBLK, 2).max(axis=0)
    return np.maximum(16, -(-counts // 16) * 16)


def _sel_layout(N16):
    """Per-block sel tile counts and column offsets (shared across cores).
    Returns (TA[NBLK], TB[NBLK], coloff[NBLK], total_cols)."""
    TA = -(-N16[:, 0] // 128)
    TB = -(-N16[:, 1] // 128)
    T = TA + TB
    coloff = np.zeros(NBLK, np.int64)
    coloff[1:] = np.cumsum(T[:-1]) * 128
    return TA, TB, coloff, int(T.sum() * 128)


def _prep_graph(src, dst, w, G_A, G_B, N16, calls, qcols):
    """Per-core gather index arrays and host-built one-hot sel arrays."""
    TA, TB, coloff, selcols = _sel_layout(N16)
    core_all = dst // PER_CORE
    out = []
    for k in range(NCORES):
        m = core_all == k
        s, ww = src[m], w[m]
        rel = dst[m] - k * PER_CORE
        blk = rel // 128
        slot = rel % 128
        cls = (s >= SPLIT).astype(np.int64)
        order = np.lexsort((s, cls, blk))
        s, ww, blk, slot, cls = s[order], ww[order], blk[order], slot[order], cls[order]

        idxA = np.zeros((NBLK, G_A * 128), np.int16)
        idxB = np.zeros((NBLK, G_B * 128), np.int16)
        key = blk * 2 + cls
        cnt = np.bincount(key, minlength=NBLK * 2).reshape(NBLK, 2)
        assert cnt[:, 0].max() <= G_A * 128 and cnt[:, 1].max() <= G_B * 128
        starts = np.concatenate([[0], np.cumsum(cnt.ravel())])
        idxQ = np.zeros((128, qcols), np.int16)

        # position of each edge within its (block, class) run
        pos_in_run = np.arange(len(s)) - starts[key]
        # sel[partition, col] = w; col = coloff[blk] + (tile + TA[blk]*isB)*128 + slot
        tile_i = pos_in_run // 128
        part = pos_in_run % 128
        coltile = coloff[blk] // 128 + tile_i + np.where(cls == 1, TA[blk], 0)
        # compact packed (slot, w) arrays for the on-device DVE sel build
        pos_pk = np.zeros((128, selcols // 128), np.float32)
        wv_pk = np.zeros((128, selcols // 128), np.float32)
        pos_pk[part, coltile] = slot
        wv_pk[part, coltile] = ww

        for b in range(NBLK):
            nA, nB = cnt[b, 0], cnt[b, 1]
            oA, oB = starts[b * 2], starts[b * 2 + 1]
            idxA[b, :nA] = s[oA:oA + nA]
            idxB[b, :nB] = s[oB:oB + nB] - SPLIT

        # Pack each call's idx slice into its queue's 32-partition window:
        # queue q's Q7 pair (cores 2q, 2q+1) reads only partitions
        # [32q, 32q+32), so 4 queues' calls overlay in the same columns.
        for (b, cl, r0, n, q, qoff) in calls:
            a = idxA if cl == 0 else idxB
            sl = a[b, r0:r0 + n].reshape(n // 16, 16).T  # [16, n//16]
            idxQ[32 * q:32 * q + 16, qoff:qoff + n // 16] = sl
            idxQ[32 * q + 16:32 * q + 32, qoff:qoff + n // 16] = sl

        out.append({
            "idxQ": idxQ,
            "pos": pos_pk.astype(BF16),
            "wv": wv_pk.astype(BF16),
        })
    return out


def _prep_x(x, k):
    """Blocked transposed node features for core k: [NBLK*128, F_IN] bf16
    with row b*128+i, col kt*128+j = x[k*PER_CORE + b*128 + j, kt*128 + i]."""
    xs = np.zeros((NBLK * 128, F_IN), BF16)
    xk = x[k * PER_CORE:(k + 1) * PER_CORE].astype(BF16)  # [6250, 512]
    for b in range(NBLK):
        rows = min(128, PER_CORE - b * 128)
        blkT = xk[b * 128:b * 128 + rows].T  # [512, rows]
        t = blkT.reshape(4, 128, rows)       # [kt, i, j]
        xs[b * 128:(b + 1) * 128, :] = np.transpose(
            np.pad(t, ((0, 0), (0, 0), (0, 128 - rows))), (1, 0, 2)
        ).reshape(128, 512)
    return xs


# ----------------------------------------------------------------------------
# Device program
# ----------------------------------------------------------------------------

def _chunks16(total, mx=768):
    """Split `total` (multiple of 16) gather rows into calls of <=mx rows
    (mx=768 = 48 of the ring's ~128 16-row entries, so two calls fit in a
    queue's descriptor ring and the NX can dispatch one ahead)."""
    out, r0 = [], 0
    while r0 < total:
        n = min(mx, total - r0)
        out.append((r0, n))
        r0 += n
    return out


NQUEUES = 4


def _call_schedule(N16):
    """Static per-graph gather call list: [(b, cls, r0, n, q, off)].
    Least-loaded queue assignment (queue q = Q7 core pair 2q,2q+1) and
    per-queue packed idx column offsets. The same schedule serves phases B
    and C (identical call structure), so one idx tensor covers both.
    Returns (calls, QCOLS)."""
    load = [0] * NQUEUES
    off = [0] * NQUEUES
    calls = []
    for b in range(NBLK):
        for cls in (0, 1):
            for r0, n in _chunks16(int(N16[b, cls])):
                q = min(range(NQUEUES), key=lambda i: load[i])
                load[q] += n
                calls.append((b, cls, r0, n, q, off[q]))
                off[q] += n // 16
    return calls, max(off)


def _graph_setup(nc, tc, sb, ps, dr, p, G_A, G_B, tens, consts, N_A, N_B,
                 TA, TB, coloff, calls, qcols):
    """Load resident tiles + alloc DRAM intermediates for one graph."""
    GT = G_A + G_B
    dt = mybir.dt
    ident_t, ones_t, iota_t = consts

    # resident per-graph tiles
    w1_t = sb.tile([128, 4, H1], dt.bfloat16, tag="w1")
    nc.sync.dma_start(out=w1_t[:], in_=tens[p + "W1"][:].rearrange("(a b) c -> b a c", b=128))
    w2_t = sb.tile([128, 2, H2], dt.bfloat16, tag="w2")
    nc.sync.dma_start(out=w2_t[:], in_=tens[p + "W2"][:].rearrange("(a b) c -> b a c", b=128))
    b1_t = sb.tile([1, H1], dt.bfloat16, tag="b1")
    nc.sync.dma_start(out=b1_t[:], in_=tens[p + "b1"][:])
    b2_t = sb.tile([1, H2], dt.bfloat16, tag="b2")
    nc.sync.dma_start(out=b2_t[:], in_=tens[p + "b2"][:])
    idxQ_t = sb.tile([128, qcols], dt.int16, tag="idxQ")
    nc.sync.dma_start(out=idxQ_t[:], in_=tens[p + "idxQ"][:])
    ntiles = int((TA + TB).sum())
    pos_t = sb.tile([128, ntiles], dt.bfloat16, tag="pos")
    nc.sync.dma_start(out=pos_t[:], in_=tens[p + "pos"][:])
    wv_t = sb.tile([128, ntiles], dt.bfloat16, tag="wv")
    nc.sync.dma_start(out=wv_t[:], in_=tens[p + "wv"][:])

    # DRAM intermediates
    s1_own = dr.tile([PER_CORE, H1], dt.bfloat16, tag=p + "s1o")
    s1_full = dr.tile([NODES, H1], dt.bfloat16, tag=p + "s1f", addr_space="Shared")
    s2_own = dr.tile([PER_CORE, H2], dt.bfloat16, tag=p + "s2o")
    s2_full = dr.tile([NODES, H2], dt.bfloat16, tag=p + "s2f", addr_space="Shared")

    return dict(locals())


def _phase_A(st):
    nc, sb, ps, p, tens = st["nc"], st["sb"], st["ps"], st["p"], st["tens"]
    dt = mybir.dt
    w1_t, s1_own = st["w1_t"], st["s1_own"]
    # ---- Phase A: support1 = x @ W1 (own rows), 2 blocks per xt DMA ----
    for b0 in range(0, NBLK, 2):
        nb = min(2, NBLK - b0)
        xt = sb.tile([128, 2, 4, 128], dt.bfloat16, tag="xt", bufs=3)
        nc.sync.dma_start(
            out=xt[:, :nb, :, :],
            in_=tens[p + "xT"][b0 * 128:(b0 + nb) * 128, :]
                .rearrange("(t p) (a c) -> p t a c", p=128, a=4),
        )
        for t in range(nb):
            b = b0 + t
            rows = min(128, PER_CORE - b * 128)
            acc = ps.tile([128, H1], dt.float32, tag="acc256", bufs=3)
            for kt in range(4):
                nc.tensor.matmul(acc[:], lhsT=xt[:, t, kt, :], rhs=w1_t[:, kt, :],
                                 start=(kt == 0), stop=(kt == 3))
            s1sb = sb.tile([128, H1], dt.bfloat16, tag="s1sb", bufs=3)
            nc.vector.tensor_copy(out=s1sb[:], in_=acc[:])
            nc.sync.dma_start(out=s1_own[b * 128:b * 128 + rows, :], in_=s1sb[:rows, :])

    nc.gpsimd.collective_compute(
        "AllGather", mybir.AluOpType.bypass,
        replica_groups=[list(range(NCORES))],
        ins=[s1_own.opt()], outs=[st["s1_full"].opt()],
    )


def _phase_B_block(st, b, do_memset):
    """One phase-B block: gathers + DVE sel build + PE scatter + relu.
    Returns the h tile for the deferred tail."""
    nc, sb, ps = st["nc"], st["sb"], st["ps"]
    dt = mybir.dt
    G_A, G_B, GT = st["G_A"], st["G_B"], st["GT"]
    TA, TB, coloff = st["TA"], st["TB"], st["coloff"]
    ones_t, iota_t = st["ones_t"], st["iota_t"]
    s1_full, b1_t = st["s1_full"], st["b1_t"]
    pos_t, wv_t = st["pos_t"], st["wv_t"]
    idxQ_t, nregs = st["idxQ_t"], st["nregs"]

    ta, tb = int(TA[b]), int(TB[b])
    msgsA = sb.tile([128, G_A, H1], dt.bfloat16, tag="mA", bufs=5, name="msgsA")
    msgsB = sb.tile([128, G_B, H1], dt.bfloat16, tag="mB", bufs=5, name="msgsB")
    if do_memset:
        nc.vector.memset(msgsA[:], 0.0)
        nc.vector.memset(msgsB[:], 0.0)
    for (cl, r0, n, q, qoff) in st["calls_by_block"][b]:
        m, src_ap = (msgsA, s1_full[:]) if cl == 0 else (msgsB, s1_full[SPLIT:, :])
        g0, g1 = r0 // 128, (r0 + n + 127) // 128
        nc.gpsimd.dma_gather(
            m[:, g0:g1, :], src_ap,
            idxQ_t[:, qoff:qoff + n // 16],
            n, nregs[n], H1, single_packet=False, queue_num=q)

    # build sel = onehot(slot) * w on the vector engine
    toff = int(coloff[b]) // 128
    posb = pos_t[:, toff:toff + ta + tb]
    wvb = wv_t[:, toff:toff + ta + tb]
    ia = iota_t[:, :]
    iota_b = bass.AP(tensor=ia.tensor, offset=ia.offset,
                     ap=[ia.ap[0], [0, ta + tb], ia.ap[1]])
    selt = sb.tile([128, GT, 128], dt.bfloat16, tag="sel", bufs=3, name="selt")
    nc.vector.tensor_tensor(out=selt[:, :ta + tb, :], in0=iota_b,
                            in1=posb.to_broadcast([128, ta + tb, 128]),
                            op=mybir.AluOpType.is_equal)
    nc.vector.tensor_tensor(out=selt[:, :ta + tb, :], in0=selt[:, :ta + tb, :],
                            in1=wvb.to_broadcast([128, ta + tb, 128]),
                            op=mybir.AluOpType.mult)

    acc = ps.tile([128, H1], dt.float32, tag="acc256", bufs=3, name="accB")
    nc.tensor.matmul(acc[:], lhsT=ones_t[:], rhs=b1_t[:], start=True, stop=False)
    for c in range(ta):
        nc.tensor.matmul(acc[:], lhsT=selt[:, c, :], rhs=msgsA[:, c, :],
                         start=False, stop=False)
    for c in range(tb):
        nc.tensor.matmul(acc[:], lhsT=selt[:, ta + c, :], rhs=msgsB[:, c, :],
                         start=False, stop=(c == tb - 1))

    h_bf = sb.tile([128, H1], dt.bfloat16, tag="hbf", bufs=3, name="h_bf")
    nc.scalar.activation(h_bf[:], acc[:], mybir.ActivationFunctionType.Relu)
    return h_bf


def _phase_B_tail(st, b, h_bf):
    """Deferred phase-B tail: h -> transposes -> @W2 -> s2_own row write."""
    nc, sb, ps = st["nc"], st["sb"], st["ps"]
    dt = mybir.dt
    ident_t, w2_t, s2_own = st["ident_t"], st["w2_t"], st["s2_own"]
    rows = min(128, PER_CORE - b * 128)
    sp2 = ps.tile([128, H2], dt.float32, tag="acc128", bufs=3, name="sp2")
    tps = []
    for half in range(2):
        tp = ps.tile([128, 128], dt.bfloat16, tag="tp", name="tp")
        nc.tensor.transpose(out=tp[:], in_=h_bf[:, half * 128:(half + 1) * 128],
                            identity=ident_t[:])
        tps.append(tp)
    hTs = []
    for half in range(2):
        hT = sb.tile([128, 128], dt.bfloat16, tag="hT", bufs=4, name="hT")
        nc.vector.tensor_copy(out=hT[:], in_=tps[half][:])
        hTs.append(hT)
    for half in range(2):
        nc.tensor.matmul(sp2[:], lhsT=hTs[half][:], rhs=w2_t[:, half, :],
                         start=(half == 0), stop=(half == 1))
    s2sb = sb.tile([128, H2], dt.bfloat16, tag="s2sb", name="s2sb")
    nc.vector.tensor_copy(out=s2sb[:], in_=sp2[:])
    nc.sync.dma_start(out=s2_own[b * 128:b * 128 + rows, :], in_=s2sb[:rows, :])


def _phase_C_block(st, b):
    """One phase-C block: gathers + DVE sel build + PE scatter + out write."""
    nc, sb, ps, tens = st["nc"], st["sb"], st["ps"], st["tens"]
    dt = mybir.dt
    p = st["p"]
    G_A, G_B, GT = st["G_A"], st["G_B"], st["GT"]
    TA, TB, coloff = st["TA"], st["TB"], st["coloff"]
    ones_t, iota_t = st["ones_t"], st["iota_t"]
    s2_full, b2_t = st["s2_full"], st["b2_t"]
    pos_t, wv_t = st["pos_t"], st["wv_t"]
    idxQ_t, nregs = st["idxQ_t"], st["nregs"]

    rows = min(128, PER_CORE - b * 128)
    ta, tb = int(TA[b]), int(TB[b])
    msgsA = sb.tile([128, G_A, H2], dt.bfloat16, tag="mA", bufs=5, name="msgsA")
    msgsB = sb.tile([128, G_B, H2], dt.bfloat16, tag="mB", bufs=5, name="msgsB")
    for (cl, r0, n, q, qoff) in st["calls_by_block"][b]:
        m, src_ap = (msgsA, s2_full[:]) if cl == 0 else (msgsB, s2_full[SPLIT:, :])
        g0, g1 = r0 // 128, (r0 + n + 127) // 128
        nc.gpsimd.dma_gather(
            m[:, g0:g1, :], src_ap,
            idxQ_t[:, qoff:qoff + n // 16],
            n, nregs[n], H2, single_packet=False, queue_num=q)

    toff = int(coloff[b]) // 128
    posb = pos_t[:, toff:toff + ta + tb]
    wvb = wv_t[:, toff:toff + ta + tb]
    ia = iota_t[:, :]
    iota_b = bass.AP(tensor=ia.tensor, offset=ia.offset,
                     ap=[ia.ap[0], [0, ta + tb], ia.ap[1]])
    selt = sb.tile([128, GT, 128], dt.bfloat16, tag="sel", bufs=3, name="selt")
    nc.vector.tensor_tensor(out=selt[:, :ta + tb, :], in0=iota_b,
                            in1=posb.to_broadcast([128, ta + tb, 128]),
                            op=mybir.AluOpType.is_equal)
    nc.vector.tensor_tensor(out=selt[:, :ta + tb, :], in0=selt[:, :ta + tb, :],
                            in1=wvb.to_broadcast([128, ta + tb, 128]),
                            op=mybir.AluOpType.mult)

    acc = ps.tile([128, H2], dt.float32, tag="acc128", bufs=3, name="accC")
    nc.tensor.matmul(acc[:], lhsT=ones_t[:], rhs=b2_t[:], start=True, stop=False)
    for c in range(ta):
        nc.tensor.matmul(acc[:], lhsT=selt[:, c, :], rhs=msgsA[:, c, :],
                         start=False, stop=False)
    for c in range(tb):
        nc.tensor.matmul(acc[:], lhsT=selt[:, ta + c, :], rhs=msgsB[:, c, :],
                         start=False, stop=(c == tb - 1))

    ob = sb.tile([128, H2], dt.float32, tag="ob", name="ob")
    nc.vector.tensor_copy(out=ob[:], in_=acc[:])
    nc.sync.dma_start(out=tens[p + "out"][b * 128:b * 128 + rows, :],
                      in_=ob[:rows, :])


def _ag2(st):
    st["nc"].gpsimd.collective_compute(
        "AllGather", mybir.AluOpType.bypass,
        replica_groups=[list(range(NCORES))],
        ins=[st["s2_own"].opt()], outs=[st["s2_full"].opt()],
    )


def _phase_BC(std, sts):
    """Phases B and C with graphs d and s interleaved at block level, d
    leading by LEAD blocks. Stalls in one graph's dependency chain hide
    under the other's work, and each graph's AllGather (issued after its
    last B block) completes under the other graph's remaining stream."""
    nmemset = [0]

    def run_B(st):
        prev = None
        for b in range(NBLK):
            do_ms = nmemset[0] < 5
            nmemset[0] += 1
            h = _phase_B_block(st, b, do_ms)
            if prev is not None:
                _phase_B_tail(st, *prev)
            prev = (b, h)
        _phase_B_tail(st, *prev)
        _ag2(st)

    run_B(std)
    run_B(sts)
    for b in range(NBLK):
        _phase_C_block(std, b)
    for b in range(NBLK):
        _phase_C_block(sts, b)


def _build_program(GAd, GBd, GAs, GBs, N16):
    dt = mybir.dt
    nc = bacc.Bacc("TRN2", target_bir_lowering=False, debug=False,
                   num_devices=NCORES, num_swdge_queues=NQUEUES)
    tens = {}

    def inp(name, shape, dtype):
        tens[name] = nc.dram_tensor(name, shape, dtype, kind="ExternalInput")

    layouts = {}
    for p, (GA, GB) in (("d", (GAd, GBd)), ("s", (GAs, GBs))):
        layouts[p] = list(_sel_layout(N16[p]))
        inp(p + "xT", [NBLK * 128, F_IN], dt.bfloat16)
        inp(p + "W1", [F_IN, H1], dt.bfloat16)
        inp(p + "W2", [H1, H2], dt.bfloat16)
        inp(p + "b1", [1, H1], dt.bfloat16)
        inp(p + "b2", [1, H2], dt.bfloat16)
        calls, qcols = _call_schedule(N16[p])
        layouts[p] = layouts[p] + [calls, qcols]
        inp(p + "idxQ", [128, qcols], dt.int16)
        inp(p + "pos", [128, layouts[p][3] // 128], dt.bfloat16)
        inp(p + "wv", [128, layouts[p][3] // 128], dt.bfloat16)
        tens[p + "out"] = nc.dram_tensor(p + "out", [PER_CORE, H2], dt.float32,
                                         kind="ExternalOutput")
    inp("ident", [128, 128], dt.bfloat16)
    inp("ones", [1, 128], dt.bfloat16)
    inp("iota", [128, 128], dt.bfloat16)

    with tile.TileContext(nc) as tc:
        with (
            tc.tile_pool(name="sbuf", bufs=2) as sb,
            tc.tile_pool(name="psum", bufs=2, space="PSUM") as ps,
            tc.tile_pool(name="dram", bufs=1, space="DRAM") as dr,
        ):
            ident_t = sb.tile([128, 128], dt.bfloat16, tag="ident")
            nc.sync.dma_start(out=ident_t[:], in_=tens["ident"][:])
            ones_t = sb.tile([1, 128], dt.bfloat16, tag="ones")
            nc.sync.dma_start(out=ones_t[:], in_=tens["ones"][:])
            iota_t = sb.tile([128, 128], dt.bfloat16, tag="iota")
            nc.sync.dma_start(out=iota_t[:], in_=tens["iota"][:])
            consts = (ident_t, ones_t, iota_t)
            sizes = {n for pp in ("d", "s")
                     for (_b, _c, _r, n, _q, _o) in layouts[pp][4]}
            nregs = {n: nc.gpsimd.to_reg(n) for n in sorted(sizes)}

            nd = N16["d"]
            ns_ = N16["s"]
            def mk(pp, GA, GB, nn):
                TA_, TB_, coloff_, _sc, calls_, qcols_ = layouts[pp]
                cbb = [[] for _ in range(NBLK)]
                for (b, cl, r0, n, q, qoff) in calls_:
                    cbb[b].append((cl, r0, n, q, qoff))
                st = _graph_setup(nc, tc, sb, ps, dr, pp, GA, GB, tens, consts,
                                  N_A=nn[:, 0], N_B=nn[:, 1],
                                  TA=TA_, TB=TB_, coloff=coloff_,
                                  calls=calls_, qcols=qcols_)
                st["calls_by_block"] = cbb
                st["nregs"] = nregs
                return st
            std = mk("d", GAd, GBd, nd)
            sts = mk("s", GAs, GBs, ns_)
            _phase_A(std)
            _phase_A(sts)
            _phase_BC(std, sts)
    return nc


# ----------------------------------------------------------------------------
# Entry point
# ----------------------------------------------------------------------------

def kernel(drug_x, dis_x, drug_src, drug_dst, drug_w,
           dis_src, dis_dst, dis_w,
           W1d, b1d, W2d, b2d, W1s, b1s, W2s, b2s,
           _run_opts=None):
    graphs = {
        "d": (drug_x, drug_src, drug_dst, drug_w, W1d, b1d, W2d, b2d),
        "s": (dis_x, dis_src, dis_dst, dis_w, W1s, b1s, W2s, b2s),
    }
    G = {}
    N16 = {}
    preps = {}
    for p, (x, src, dst, w, W1, b1, W2, b2) in graphs.items():
        src = np.asarray(src); dst = np.asarray(dst); w = np.asarray(w)
        mA, mB = _analyze(src, dst)
        GA, GB = -(-mA // 128), -(-mB // 128)
        G[p] = (GA, GB)
        N16[p] = _analyze_blocks(src, dst)
        calls, qcols = _call_schedule(N16[p])
        preps[p] = _prep_graph(src, dst, w, GA, GB, N16[p], calls, qcols)

    nc = _build_program(G["d"][0], G["d"][1], G["s"][0], G["s"][1], N16)
    nc.compile()

    base = {
        "ident": np.eye(128, dtype=np.float32).astype(BF16),
        "ones": np.ones((1, 128), BF16),
        "iota": np.tile(np.arange(128, dtype=np.float32)[None, :].astype(BF16), (128, 1)),
    }
    for p, (x, src, dst, w, W1, b1, W2, b2) in graphs.items():
        base[p + "W1"] = np.asarray(W1).astype(BF16)
        base[p + "W2"] = np.asarray(W2).astype(BF16)
        base[p + "b1"] = np.asarray(b1).astype(BF16)[None, :]
        base[p + "b2"] = np.asarray(b2).astype(BF16)[None, :]

    in_maps = []
    for k in range(NCORES):
        m = dict(base)
        for p, (x, *_rest) in graphs.items():
            m[p + "xT"] = _prep_x(np.asarray(x), k)
            m.update({p + n: preps[p][k][n] for n in ("idxQ", "pos", "wv")})
        in_maps.append(m)

    res = run_bass_kernel_spmd(nc, in_maps, core_ids=list(range(NCORES)),
                               **(_run_opts or {}))
    emb1 = np.concatenate([res.results[k]["dout"] for k in range(NCORES)], axis=0)
    emb2 = np.concatenate([res.results[k]["sout"] for k in range(NCORES)], axis=0)
    if _run_opts:
        kernel.last_results = res
    return emb1, emb2


# revision 26
# speedup vs baseline: 1.0409x; 1.0409x over previous
"""Trainium2 Bass kernel for the 2-graph GCN (nn_Net_39041252721058).

Strategy (8 NeuronCores, SPMD single program):
  - Core k owns dst nodes [k*6250, (k+1)*6250). All edges with dst in that
    range are processed by core k, grouped by 128-node dst blocks.
  - Layer math uses the linearity of spmm: project first (x@W1 row-sharded,
    AllGather of the projected bf16 table), then per-edge gather rows of the
    table with SWDGE dma_gather (int16 indices; src split in two classes at
    row 32768 so indices fit), then segment-sum via one-hot matmuls on the
    tensor engine accumulating in PSUM (edges on the contraction axis).
  - The one-hot dst-slot selection matrices (sel = onehot(slot)*w) are built
    per block on the vector engine (is_equal + mult) from compact packed
    (slot, w) arrays; shipping pre-expanded sel from DRAM was tried and is
    net-negative: it loads the SDMA engines (~41 ns/descriptor, the
    co-bottleneck) with 99.3%-zeros traffic.
  - h = relu(agg + b1) on the scalar engine; the PE tail (transposes + h@W2)
    for block b is deferred until after block b+1's sel matmuls so the PE
    stream stays continuous (p-state ramps to 2.4 GHz only after ~3 us of
    uninterrupted execution; gaps reset it to 0.65/1.2 GHz).
  - Phases: A_d, AG1_d, A_s (hides AG1_d), AG1_s, B_d (hides AG1_s), AG2_d,
    B_s (hides AG2_d), AG2_s, C_d (hides AG2_s), C_s.

Perf model (measured on trn2, see git-less history in this session):
  - The kernel is bound by the per-edge gather stream: 2 graphs x 2 layers
    x 200K edges/core = ~827K gathered rows/core (incl. ~3.5% ceil-16 pad).
  - SWDGE descriptor generation runs ~8.4 ns/row per Q7 core pair. It is
    parallelized across all 4 SWDGE queues (bass num_swdge_queues=4; queue q
    runs on Q7 cores 2q,2q+1 with its own full-size descriptor ring in its
    own 32-partition group). Measured queue scaling on an isolated
    microbench (bench_queues.py): 1q=9.1, 2q=5.0, 4q=3.0 ns/row -- the 4q
    cap is partly SDMA-side (per-descriptor processing ~25-41 ns across 16
    engines; 512B rows are descriptor-dominated, 1KB rows 34% cheaper/byte).
  - Gather calls are chunked to 768 rows so two calls fit in a queue's
    descriptor ring (the NX await_space blocks in-order dispatch when a
    ring is full); call sizes are load-balanced across queues host-side
    (greedy least-loaded; naive round-robin gave a 2x per-queue imbalance).
  - Each call's idx slice is stored only in its queue's 32-partition window
    (4 queues overlay in the same columns), shrinking resident idx SBUF 4x.
  - num_idxs registers are hoisted (one per distinct call size) so gather
    dispatch does not pay a MOVE per call.
  - msgs tiles are 5-deep and sel/psum 3-deep so gathers run ~5 blocks
    ahead of the PE; block-level d/s interleaving was tried and is neutral
    (shared buffer pool halves per-graph lookahead).
  - Single-run best: 2.95 ms (baseline 8.76 ms). Remaining time: ~0.25 ms
    phase-A+AG1 head (AllGather chunking fails: a Shared DRAM tensor allows
    only one writer instruction), and B/C run ~10% above the isolated
    gather floor (cross-engine semaphore latency).

Correctness details: pads carry sel=0 so the one-hot multiply zeroes them;
message buffers are memset once on first use so never-gathered pad slots
cannot inject NaN (0*NaN = NaN would poison PSUM); phase-C reuses the same
buffers after phase B filled them with finite values, so only the first 5
phase-B allocations are memset.
"""
import numpy as np
import ml_dtypes

import concourse.bass as bass
import concourse.bacc as bacc
import concourse.mybir as mybir
import concourse.tile as tile
from concourse.bass_utils import run_bass_kernel_spmd

NCORES = 8
NODES = 50000
PER_CORE = NODES // NCORES           # 6250
NBLK = (PER_CORE + 127) // 128       # 49 (last block has 106 nodes)
# Gather tables are AllGathered in two chunks so phase B can start on chunk A
# while chunk B is still in flight: class A = rows r < CHUNK1 of each rank's
# shard (table s1fA [8*CHUNK1, H]), class B = the rest (s1fB [8*CHUNK2, H]).
# Both tables stay under 32768 rows so int16 gather indices fit.
CHUNK1 = 3328                        # 26 blocks of 128
CHUNK2 = PER_CORE - CHUNK1           # 2922
AG1_AT = 24                          # issue chunk-A AllGather after this b0
F_IN = 512
H1 = 256
H2 = 128

BF16 = ml_dtypes.bfloat16


# ----------------------------------------------------------------------------
# Host-side edge preprocessing
# ----------------------------------------------------------------------------

def _cls_of(src):
    return ((src % PER_CORE) >= CHUNK1).astype(np.int64)


def _remap_idx(src):
    """Index into the chunk table (s1fA/s1fB) for each src node."""
    rank = src // PER_CORE
    r = src % PER_CORE
    return np.where(r >= CHUNK1, rank * CHUNK2 + (r - CHUNK1),
                    rank * CHUNK1 + r).astype(np.int16)


def _analyze(src, dst):
    core = dst // PER_CORE
    blk = (dst % PER_CORE) // 128
    cls = _cls_of(src)
    key = (core * NBLK + blk) * 2 + cls
    counts = np.bincount(key, minlength=NCORES * NBLK * 2).reshape(-1, 2)
    return int(counts[:, 0].max()), int(counts[:, 1].max())


def _analyze_blocks(src, dst):
    """Per-block max-over-cores counts, ceil-16, per class: [NBLK, 2] int."""
    core = dst // PER_CORE
    blk = (dst % PER_CORE) // 128
    cls = _cls_of(src)
    key = (core * NBLK + blk) * 2 + cls
    counts = np.bincount(key, minlength=NCORES * NBLK * 2)
    counts = counts.reshape(NCORES, NBLK, 2).max(axis=0)
    return np.maximum(16, -(-counts // 16) * 16)


def _sel_layout(N16):
    """Per-block sel tile counts and column offsets (shared across cores).
    Returns (TA[NBLK], TB[NBLK], coloff[NBLK], total_cols)."""
    TA = -(-N16[:, 0] // 128)
    TB = -(-N16[:, 1] // 128)
    T = TA + TB
    coloff = np.zeros(NBLK, np.int64)
    coloff[1:] = np.cumsum(T[:-1]) * 128
    return TA, TB, coloff, int(T.sum() * 128)


def _prep_graph(src, dst, w, G_A, G_B, N16, calls, qcols):
    """Per-core gather index arrays and host-built one-hot sel arrays."""
    TA, TB, coloff, selcols = _sel_layout(N16)
    core_all = dst // PER_CORE
    out = []
    for k in range(NCORES):
        m = core_all == k
        s, ww = src[m], w[m]
        rel = dst[m] - k * PER_CORE
        blk = rel // 128
        slot = rel % 128
        cls = _cls_of(s)
        order = np.lexsort((s, cls, blk))
        s, ww, blk, slot, cls = s[order], ww[order], blk[order], slot[order], cls[order]
        s = _remap_idx(s)

        idxA = np.zeros((NBLK, G_A * 128), np.int16)
        idxB = np.zeros((NBLK, G_B * 128), np.int16)
        key = blk * 2 + cls
        cnt = np.bincount(key, minlength=NBLK * 2).reshape(NBLK, 2)
        assert cnt[:, 0].max() <= G_A * 128 and cnt[:, 1].max() <= G_B * 128
        starts = np.concatenate([[0], np.cumsum(cnt.ravel())])
        idxQ = np.zeros((128, qcols), np.int16)

        # position of each edge within its (block, class) run
        pos_in_run = np.arange(len(s)) - starts[key]
        # sel[partition, col] = w; col = coloff[blk] + (tile + TA[blk]*isB)*128 + slot
        tile_i = pos_in_run // 128
        part = pos_in_run % 128
        coltile = coloff[blk] // 128 + tile_i + np.where(cls == 1, TA[blk], 0)
        # compact packed (slot, w) arrays for the on-device DVE sel build
        pos_pk = np.zeros((128, selcols // 128), np.float32)
        wv_pk = np.zeros((128, selcols // 128), np.float32)
        pos_pk[part, coltile] = slot
        wv_pk[part, coltile] = ww

        for b in range(NBLK):
            nA, nB = cnt[b, 0], cnt[b, 1]
            oA, oB = starts[b * 2], starts[b * 2 + 1]
            idxA[b, :nA] = s[oA:oA + nA]
            idxB[b, :nB] = s[oB:oB + nB]

        # Pack each call's idx slice into its queue's 32-partition window:
        # queue q's Q7 pair (cores 2q, 2q+1) reads only partitions
        # [32q, 32q+32), so 4 queues' calls overlay in the same columns.
        for (b, cl, r0, n, q, qoff) in calls:
            a = idxA if cl == 0 else idxB
            sl = a[b, r0:r0 + n].reshape(n // 16, 16).T  # [16, n//16]
            idxQ[32 * q:32 * q + 16, qoff:qoff + n // 16] = sl
            idxQ[32 * q + 16:32 * q + 32, qoff:qoff + n // 16] = sl

        out.append({
            "idxQ": idxQ,
            "pos": pos_pk.astype(BF16),
            "wv": wv_pk.astype(BF16),
        })
    return out


def _prep_x(x, k):
    """Blocked transposed node features for core k: [NBLK*128, F_IN] bf16
    with row b*128+i, col kt*128+j = x[k*PER_CORE + b*128 + j, kt*128 + i]."""
    xs = np.zeros((NBLK * 128, F_IN), BF16)
    xk = x[k * PER_CORE:(k + 1) * PER_CORE].astype(BF16)  # [6250, 512]
    for b in range(NBLK):
        rows = min(128, PER_CORE - b * 128)
        blkT = xk[b * 128:b * 128 + rows].T  # [512, rows]
        t = blkT.reshape(4, 128, rows)       # [kt, i, j]
        xs[b * 128:(b + 1) * 128, :] = np.transpose(
            np.pad(t, ((0, 0), (0, 0), (0, 128 - rows))), (1, 0, 2)
        ).reshape(128, 512)
    return xs


# ----------------------------------------------------------------------------
# Device program
# ----------------------------------------------------------------------------

def _chunks16(total, mx=768):
    """Split `total` (multiple of 16) gather rows into calls of <=mx rows
    (mx=768 = 48 of the ring's ~128 16-row entries, so two calls fit in a
    queue's descriptor ring and the NX can dispatch one ahead)."""
    out, r0 = [], 0
    while r0 < total:
        n = min(mx, total - r0)
        out.append((r0, n))
        r0 += n
    return out


NQUEUES = 4


def _call_schedule(N16):
    """Static per-graph gather call list: [(b, cls, r0, n, q, off)].
    Least-loaded queue assignment (queue q = Q7 core pair 2q,2q+1) and
    per-queue packed idx column offsets. The same schedule serves phases B
    and C (identical call structure), so one idx tensor covers both.
    Returns (calls, QCOLS)."""
    load = [0] * NQUEUES
    off = [0] * NQUEUES
    calls = []
    for b in range(NBLK):
        for cls in (0, 1):
            for r0, n in _chunks16(int(N16[b, cls])):
                q = min(range(NQUEUES), key=lambda i: load[i])
                load[q] += n
                calls.append((b, cls, r0, n, q, off[q]))
                off[q] += n // 16
    return calls, max(off)


def _graph_setup(nc, tc, sb, ps, dr, p, G_A, G_B, tens, consts, N_A, N_B,
                 TA, TB, coloff, calls, qcols):
    """Load resident tiles + alloc DRAM intermediates for one graph."""
    GT = G_A + G_B
    dt = mybir.dt
    ident_t, ones_t, iota_t = consts

    # resident per-graph tiles
    w1_t = sb.tile([128, 4, H1], dt.bfloat16, tag="w1")
    nc.sync.dma_start(out=w1_t[:], in_=tens[p + "W1"][:].rearrange("(a b) c -> b a c", b=128))
    w2_t = sb.tile([128, 2, H2], dt.bfloat16, tag="w2")
    nc.sync.dma_start(out=w2_t[:], in_=tens[p + "W2"][:].rearrange("(a b) c -> b a c", b=128))
    b1_t = sb.tile([1, H1], dt.bfloat16, tag="b1")
    nc.sync.dma_start(out=b1_t[:], in_=tens[p + "b1"][:])
    b2_t = sb.tile([1, H2], dt.bfloat16, tag="b2")
    nc.sync.dma_start(out=b2_t[:], in_=tens[p + "b2"][:])
    idxQ_t = sb.tile([128, qcols], dt.int16, tag="idxQ")
    nc.sync.dma_start(out=idxQ_t[:], in_=tens[p + "idxQ"][:])
    ntiles = int((TA + TB).sum())
    pos_t = sb.tile([128, ntiles], dt.bfloat16, tag="pos")
    nc.sync.dma_start(out=pos_t[:], in_=tens[p + "pos"][:])
    wv_t = sb.tile([128, ntiles], dt.bfloat16, tag="wv")
    nc.sync.dma_start(out=wv_t[:], in_=tens[p + "wv"][:])

    # DRAM intermediates: the gather tables are split into two chunk
    # tensors (class A/B) so each can be AllGathered independently
    s1_own = dr.tile([PER_CORE, H1], dt.bfloat16, tag=p + "s1o")
    s1fA = dr.tile([NCORES * CHUNK1, H1], dt.bfloat16, tag=p + "s1fA",
                   addr_space="Shared")
    s1fB = dr.tile([NCORES * CHUNK2, H1], dt.bfloat16, tag=p + "s1fB",
                   addr_space="Shared")
    s2_own = dr.tile([PER_CORE, H2], dt.bfloat16, tag=p + "s2o")
    s2fA = dr.tile([NCORES * CHUNK1, H2], dt.bfloat16, tag=p + "s2fA",
                   addr_space="Shared")
    s2fB = dr.tile([NCORES * CHUNK2, H2], dt.bfloat16, tag=p + "s2fB",
                   addr_space="Shared")

    return dict(locals())


def _phase_A(st):
    nc, sb, ps, p, tens = st["nc"], st["sb"], st["ps"], st["p"], st["tens"]
    dt = mybir.dt
    w1_t, s1_own = st["w1_t"], st["s1_own"]
    # ---- Phase A: support1 = x @ W1 (own rows), 2 blocks per xt DMA ----
    for b0 in range(0, NBLK, 2):
        nb = min(2, NBLK - b0)
        xt = sb.tile([128, 2, 4, 128], dt.bfloat16, tag="xt", bufs=3)
        nc.sync.dma_start(
            out=xt[:, :nb, :, :],
            in_=tens[p + "xT"][b0 * 128:(b0 + nb) * 128, :]
                .rearrange("(t p) (a c) -> p t a c", p=128, a=4),
        )
        for t in range(nb):
            b = b0 + t
            rows = min(128, PER_CORE - b * 128)
            acc = ps.tile([128, H1], dt.float32, tag="acc256", bufs=3)
            for kt in range(4):
                nc.tensor.matmul(acc[:], lhsT=xt[:, t, kt, :], rhs=w1_t[:, kt, :],
                                 start=(kt == 0), stop=(kt == 3))
            s1sb = sb.tile([128, H1], dt.bfloat16, tag="s1sb", bufs=3)
            nc.vector.tensor_copy(out=s1sb[:], in_=acc[:])
            nc.sync.dma_start(out=s1_own[b * 128:b * 128 + rows, :], in_=s1sb[:rows, :])
        # chunk-A AllGather ships early so phase B's class-A gathers can
        # start while the chunk-B AllGather is still in flight
        if b0 == AG1_AT:
            nc.gpsimd.collective_compute(
                "AllGather", mybir.AluOpType.bypass,
                replica_groups=[list(range(NCORES))],
                ins=[s1_own[0:CHUNK1, :].opt()], outs=[st["s1fA"].opt()],
            )

    nc.gpsimd.collective_compute(
        "AllGather", mybir.AluOpType.bypass,
        replica_groups=[list(range(NCORES))],
        ins=[s1_own[CHUNK1:PER_CORE, :].opt()], outs=[st["s1fB"].opt()],
    )


def _emit_gathers(st, b, msgsA, msgsB, tabA, tabB, H, only_cls=None):
    nc = st["nc"]
    idxQ_t, nregs = st["idxQ_t"], st["nregs"]
    for (cl, r0, n, q, qoff) in st["calls_by_block"][b]:
        if only_cls is not None and cl != only_cls:
            continue
        m, src_ap = (msgsA, tabA[:]) if cl == 0 else (msgsB, tabB[:])
        g0, g1 = r0 // 128, (r0 + n + 127) // 128
        nc.gpsimd.dma_gather(
            m[:, g0:g1, :], src_ap,
            idxQ_t[:, qoff:qoff + n // 16],
            n, nregs[n], H, single_packet=False, queue_num=q)


def _phase_B_block(st, b, do_memset, preA=None):
    """One phase-B block: gathers + DVE sel build + PE scatter + relu.
    Returns the h tile for the deferred tail. preA: msgsA tile whose
    class-A gathers were already emitted (head pre-emission)."""
    nc, sb, ps = st["nc"], st["sb"], st["ps"]
    dt = mybir.dt
    G_A, G_B, GT = st["G_A"], st["G_B"], st["GT"]
    TA, TB, coloff = st["TA"], st["TB"], st["coloff"]
    ones_t, iota_t = st["ones_t"], st["iota_t"]
    b1_t = st["b1_t"]
    pos_t, wv_t = st["pos_t"], st["wv_t"]

    ta, tb = int(TA[b]), int(TB[b])
    msgsB = sb.tile([128, G_B, H1], dt.bfloat16, tag="mB", bufs=5, name="msgsB")
    if do_memset:
        nc.vector.memset(msgsB[:], 0.0)
    if preA is not None:
        msgsA = preA
        _emit_gathers(st, b, msgsA, msgsB, st["s1fA"], st["s1fB"], H1, only_cls=1)
    else:
        msgsA = sb.tile([128, G_A, H1], dt.bfloat16, tag="mA", bufs=5, name="msgsA")
        if do_memset:
            nc.vector.memset(msgsA[:], 0.0)
        _emit_gathers(st, b, msgsA, msgsB, st["s1fA"], st["s1fB"], H1)

    # build sel = onehot(slot) * w on the vector engine
    toff = int(coloff[b]) // 128
    posb = pos_t[:, toff:toff + ta + tb]
    wvb = wv_t[:, toff:toff + ta + tb]
    ia = iota_t[:, :]
    iota_b = bass.AP(tensor=ia.tensor, offset=ia.offset,
                     ap=[ia.ap[0], [0, ta + tb], ia.ap[1]])
    selt = sb.tile([128, GT, 128], dt.bfloat16, tag="sel", bufs=3, name="selt")
    nc.vector.tensor_tensor(out=selt[:, :ta + tb, :], in0=iota_b,
                            in1=posb.to_broadcast([128, ta + tb, 128]),
                            op=mybir.AluOpType.is_equal)
    nc.vector.tensor_tensor(out=selt[:, :ta + tb, :], in0=selt[:, :ta + tb, :],
                            in1=wvb.to_broadcast([128, ta + tb, 128]),
                            op=mybir.AluOpType.mult)

    acc = ps.tile([128, H1], dt.float32, tag="acc256", bufs=3, name="accB")
    nc.tensor.matmul(acc[:], lhsT=ones_t[:], rhs=b1_t[:], start=True, stop=False)
    for c in range(ta):
        nc.tensor.matmul(acc[:], lhsT=selt[:, c, :], rhs=msgsA[:, c, :],
                         start=False, stop=False)
    for c in range(tb):
        nc.tensor.matmul(acc[:], lhsT=selt[:, ta + c, :], rhs=msgsB[:, c, :],
                         start=False, stop=(c == tb - 1))

    h_bf = sb.tile([128, H1], dt.bfloat16, tag="hbf", bufs=3, name="h_bf")
    nc.scalar.activation(h_bf[:], acc[:], mybir.ActivationFunctionType.Relu)
    return h_bf


def _phase_B_tail(st, b, h_bf):
    """Deferred phase-B tail: h -> transposes -> @W2 -> s2_own row write."""
    nc, sb, ps = st["nc"], st["sb"], st["ps"]
    dt = mybir.dt
    ident_t, w2_t, s2_own = st["ident_t"], st["w2_t"], st["s2_own"]
    rows = min(128, PER_CORE - b * 128)
    sp2 = ps.tile([128, H2], dt.float32, tag="acc128", bufs=3, name="sp2")
    tps = []
    for half in range(2):
        tp = ps.tile([128, 128], dt.bfloat16, tag="tp", name="tp")
        nc.tensor.transpose(out=tp[:], in_=h_bf[:, half * 128:(half + 1) * 128],
                            identity=ident_t[:])
        tps.append(tp)
    hTs = []
    for half in range(2):
        hT = sb.tile([128, 128], dt.bfloat16, tag="hT", bufs=4, name="hT")
        nc.vector.tensor_copy(out=hT[:], in_=tps[half][:])
        hTs.append(hT)
    for half in range(2):
        nc.tensor.matmul(sp2[:], lhsT=hTs[half][:], rhs=w2_t[:, half, :],
                         start=(half == 0), stop=(half == 1))
    s2sb = sb.tile([128, H2], dt.bfloat16, tag="s2sb", name="s2sb")
    nc.vector.tensor_copy(out=s2sb[:], in_=sp2[:])
    nc.sync.dma_start(out=s2_own[b * 128:b * 128 + rows, :], in_=s2sb[:rows, :])


def _phase_C_block(st, b):
    """One phase-C block: gathers + DVE sel build + PE scatter + out write."""
    nc, sb, ps, tens = st["nc"], st["sb"], st["ps"], st["tens"]
    dt = mybir.dt
    p = st["p"]
    G_A, G_B, GT = st["G_A"], st["G_B"], st["GT"]
    TA, TB, coloff = st["TA"], st["TB"], st["coloff"]
    ones_t, iota_t = st["ones_t"], st["iota_t"]
    b2_t = st["b2_t"]
    pos_t, wv_t = st["pos_t"], st["wv_t"]

    rows = min(128, PER_CORE - b * 128)
    ta, tb = int(TA[b]), int(TB[b])
    msgsA = sb.tile([128, G_A, H2], dt.bfloat16, tag="mA", bufs=5, name="msgsA")
    msgsB = sb.tile([128, G_B, H2], dt.bfloat16, tag="mB", bufs=5, name="msgsB")
    _emit_gathers(st, b, msgsA, msgsB, st["s2fA"], st["s2fB"], H2)

    toff = int(coloff[b]) // 128
    posb = pos_t[:, toff:toff + ta + tb]
    wvb = wv_t[:, toff:toff + ta + tb]
    ia = iota_t[:, :]
    iota_b = bass.AP(tensor=ia.tensor, offset=ia.offset,
                     ap=[ia.ap[0], [0, ta + tb], ia.ap[1]])
    selt = sb.tile([128, GT, 128], dt.bfloat16, tag="sel", bufs=3, name="selt")
    nc.vector.tensor_tensor(out=selt[:, :ta + tb, :], in0=iota_b,
                            in1=posb.to_broadcast([128, ta + tb, 128]),
                            op=mybir.AluOpType.is_equal)
    nc.vector.tensor_tensor(out=selt[:, :ta + tb, :], in0=selt[:, :ta + tb, :],
                            in1=wvb.to_broadcast([128, ta + tb, 128]),
                            op=mybir.AluOpType.mult)

    acc = ps.tile([128, H2], dt.float32, tag="acc128", bufs=3, name="accC")
    nc.tensor.matmul(acc[:], lhsT=ones_t[:], rhs=b2_t[:], start=True, stop=False)
    for c in range(ta):
        nc.tensor.matmul(acc[:], lhsT=selt[:, c, :], rhs=msgsA[:, c, :],
                         start=False, stop=False)
    for c in range(tb):
        nc.tensor.matmul(acc[:], lhsT=selt[:, ta + c, :], rhs=msgsB[:, c, :],
                         start=False, stop=(c == tb - 1))

    ob = sb.tile([128, H2], dt.float32, tag="ob", name="ob")
    nc.vector.tensor_copy(out=ob[:], in_=acc[:])
    nc.sync.dma_start(out=tens[p + "out"][b * 128:b * 128 + rows, :],
                      in_=ob[:rows, :])


def _ag2a(st):
    st["nc"].gpsimd.collective_compute(
        "AllGather", mybir.AluOpType.bypass,
        replica_groups=[list(range(NCORES))],
        ins=[st["s2_own"][0:CHUNK1, :].opt()], outs=[st["s2fA"].opt()],
    )


def _ag2b(st):
    st["nc"].gpsimd.collective_compute(
        "AllGather", mybir.AluOpType.bypass,
        replica_groups=[list(range(NCORES))],
        ins=[st["s2_own"][CHUNK1:PER_CORE, :].opt()], outs=[st["s2fB"].opt()],
    )


def _phase_BC(std, sts):
    """Phases B and C with graphs d and s interleaved at block level, d
    leading by LEAD blocks. Stalls in one graph's dependency chain hide
    under the other's work, and each graph's AllGather (issued after its
    last B block) completes under the other graph's remaining stream."""
    PRE = 5

    def run_B(st, first):
        nc, sb = st["nc"], st["sb"]
        dt = mybir.dt
        G_A = st["G_A"]
        # pre-emit the first PRE blocks' class-A gathers: they depend only
        # on the chunk-A AllGather, which lands while chunk B is in flight
        preA = []
        for b in range(PRE):
            mA = sb.tile([128, G_A, H1], dt.bfloat16, tag="mA", bufs=5,
                         name="msgsA")
            if first:
                nc.vector.memset(mA[:], 0.0)
            _emit_gathers(st, b, mA, None, st["s1fA"], None, H1, only_cls=0)
            preA.append(mA)
        prev = None
        for b in range(NBLK):
            h = _phase_B_block(st, b, first and b < PRE,
                               preA=preA[b] if b < PRE else None)
            if prev is not None:
                _phase_B_tail(st, *prev)
                if prev[0] == AG1_AT + 1:
                    _ag2a(st)
            prev = (b, h)
        _phase_B_tail(st, *prev)
        _ag2b(st)

    run_B(std, True)
    run_B(sts, False)
    for b in range(NBLK):
        _phase_C_block(std, b)
    for b in range(NBLK):
        _phase_C_block(sts, b)


def _build_program(GAd, GBd, GAs, GBs, N16):
    dt = mybir.dt
    nc = bacc.Bacc("TRN2", target_bir_lowering=False, debug=False,
                   num_devices=NCORES, num_swdge_queues=NQUEUES)
    tens = {}

    def inp(name, shape, dtype):
        tens[name] = nc.dram_tensor(name, shape, dtype, kind="ExternalInput")

    layouts = {}
    for p, (GA, GB) in (("d", (GAd, GBd)), ("s", (GAs, GBs))):
        layouts[p] = list(_sel_layout(N16[p]))
        inp(p + "xT", [NBLK * 128, F_IN], dt.bfloat16)
        inp(p + "W1", [F_IN, H1], dt.bfloat16)
        inp(p + "W2", [H1, H2], dt.bfloat16)
        inp(p + "b1", [1, H1], dt.bfloat16)
        inp(p + "b2", [1, H2], dt.bfloat16)
        calls, qcols = _call_schedule(N16[p])
        layouts[p] = layouts[p] + [calls, qcols]
        inp(p + "idxQ", [128, qcols], dt.int16)
        inp(p + "pos", [128, layouts[p][3] // 128], dt.bfloat16)
        inp(p + "wv", [128, layouts[p][3] // 128], dt.bfloat16)
        tens[p + "out"] = nc.dram_tensor(p + "out", [PER_CORE, H2], dt.float32,
                                         kind="ExternalOutput")
    inp("ident", [128, 128], dt.bfloat16)
    inp("ones", [1, 128], dt.bfloat16)
    inp("iota", [128, 128], dt.bfloat16)

    with tile.TileContext(nc) as tc:
        with (
            tc.tile_pool(name="sbuf", bufs=2) as sb,
            tc.tile_pool(name="psum", bufs=2, space="PSUM") as ps,
            tc.tile_pool(name="dram", bufs=1, space="DRAM") as dr,
        ):
            ident_t = sb.tile([128, 128], dt.bfloat16, tag="ident")
            nc.sync.dma_start(out=ident_t[:], in_=tens["ident"][:])
            ones_t = sb.tile([1, 128], dt.bfloat16, tag="ones")
            nc.sync.dma_start(out=ones_t[:], in_=tens["ones"][:])
            iota_t = sb.tile([128, 128], dt.bfloat16, tag="iota")
            nc.sync.dma_start(out=iota_t[:], in_=tens["iota"][:])
            consts = (ident_t, ones_t, iota_t)
            sizes = {n for pp in ("d", "s")
                     for (_b, _c, _r, n, _q, _o) in layouts[pp][4]}
            nregs = {n: nc.gpsimd.to_reg(n) for n in sorted(sizes)}

            nd = N16["d"]
            ns_ = N16["s"]
            def mk(pp, GA, GB, nn):
                TA_, TB_, coloff_, _sc, calls_, qcols_ = layouts[pp]
                cbb = [[] for _ in range(NBLK)]
                for (b, cl, r0, n, q, qoff) in calls_:
                    cbb[b].append((cl, r0, n, q, qoff))
                st = _graph_setup(nc, tc, sb, ps, dr, pp, GA, GB, tens, consts,
                                  N_A=nn[:, 0], N_B=nn[:, 1],
                                  TA=TA_, TB=TB_, coloff=coloff_,
                                  calls=calls_, qcols=qcols_)
                st["calls_by_block"] = cbb
                st["nregs"] = nregs
                return st
            std = mk("d", GAd, GBd, nd)
            sts = mk("s", GAs, GBs, ns_)
            _phase_A(std)
            _phase_A(sts)
            _phase_BC(std, sts)
    return nc


# ----------------------------------------------------------------------------
# Entry point
# ----------------------------------------------------------------------------

def kernel(drug_x, dis_x, drug_src, drug_dst, drug_w,
           dis_src, dis_dst, dis_w,
           W1d, b1d, W2d, b2d, W1s, b1s, W2s, b2s,
           _run_opts=None):
    graphs = {
        "d": (drug_x, drug_src, drug_dst, drug_w, W1d, b1d, W2d, b2d),
        "s": (dis_x, dis_src, dis_dst, dis_w, W1s, b1s, W2s, b2s),
    }
    G = {}
    N16 = {}
    preps = {}
    for p, (x, src, dst, w, W1, b1, W2, b2) in graphs.items():
        src = np.asarray(src); dst = np.asarray(dst); w = np.asarray(w)
        mA, mB = _analyze(src, dst)
        GA, GB = -(-mA // 128), -(-mB // 128)
        G[p] = (GA, GB)
        N16[p] = _analyze_blocks(src, dst)
        calls, qcols = _call_schedule(N16[p])
        preps[p] = _prep_graph(src, dst, w, GA, GB, N16[p], calls, qcols)

    nc = _build_program(G["d"][0], G["d"][1], G["s"][0], G["s"][1], N16)
    nc.compile()

    base = {
        "ident": np.eye(128, dtype=np.float32).astype(BF16),
        "ones": np.ones((1, 128), BF16),
        "iota": np.tile(np.arange(128, dtype=np.float32)[None, :].astype(BF16), (128, 1)),
    }
    for p, (x, src, dst, w, W1, b1, W2, b2) in graphs.items():
        base[p + "W1"] = np.asarray(W1).astype(BF16)
        base[p + "W2"] = np.asarray(W2).astype(BF16)
        base[p + "b1"] = np.asarray(b1).astype(BF16)[None, :]
        base[p + "b2"] = np.asarray(b2).astype(BF16)[None, :]

    in_maps = []
    for k in range(NCORES):
        m = dict(base)
        for p, (x, *_rest) in graphs.items():
            m[p + "xT"] = _prep_x(np.asarray(x), k)
            m.update({p + n: preps[p][k][n] for n in ("idxQ", "pos", "wv")})
        in_maps.append(m)

    res = run_bass_kernel_spmd(nc, in_maps, core_ids=list(range(NCORES)),
                               **(_run_opts or {}))
    emb1 = np.concatenate([res.results[k]["dout"] for k in range(NCORES)], axis=0)
    emb2 = np.concatenate([res.results[k]["sout"] for k in range(NCORES)], axis=0)
    if _run_opts:
        kernel.last_results = res
    return emb1, emb2
